# revision 3
# baseline (speedup 1.0000x reference)
"""AWQ int4 GEMM (M=1024, K=4096, N=11008, group_size=128) on 8 TRN2 NeuronCores.

Column-parallel tensor sharding: qweight/qzeros/scales split along N across the
8 cores, x replicated, outputs concatenated.

Per-core device kernel:
  - qweight int32 k-tiles are unpacked on the DVE with the "pair trick":
    (q >> 4t) & 0x000F000F yields AWQ nibbles (2t, 2t+1) as packed int16
    pairs, so 4 fused shift+and tensor_scalar ops unpack a whole k-tile,
    int16-bitcast gives the unpacked values with zero extra work.  Columns
    land in a per-core permutation (pair-block order) that the scales are
    pre-permuted to on the host; the output is unpermuted on-chip.
  - zero-points are NOT subtracted elementwise.  Instead
        out = x @ (w4*s) - xsum_g @ (z_g * s_g)
    where xsum_g[m] = sum over group g of x[m, :].  The second term is a
    rank-32 matmul; xsum is computed on the TensorE with a one-hot
    stationary operand while x is being transposed.
  - x is cast f32->bf16 by a GPSIMD (SWDGE) casting DMA and transposed
    128x128 via the DMA xbar so K lands on partitions.
  - main GEMM: bf16 matmuls accumulating fp32 in PSUM, k-contiguous per
    m-tile chunk to keep the PE warm.
"""

import os
import sys
import types

sys.path.insert(0, "/opt/trn_rl_repo")

import numpy as np
import ml_dtypes

import concourse.bass as bass
import concourse.mybir as mybir
import concourse.tile as tile
import bass_rust as _br
from concourse.vector_clock import ScopedClock
from concourse.bass_utils import run_bass_kernel_spmd

# ---------------------------------------------------------------------------
# Walrus workaround: this toolchain rejects >1 sem wait per instruction
# (2 for InstEventSemaphore).  Tile's sem assigner can emit more; split the
# excess onto no-ops placed immediately before on the same engine.
# ---------------------------------------------------------------------------
_orig_lower = tile.TileContext._lower_ordered_insts
_wsplit_counter = [0]


def _split_waits_in_place(nc, insts):
    new_list = []
    for inst in insts:
        si = inst.sync_info
        cap = 2 if isinstance(inst, mybir.InstEventSemaphore) else 1
        if si is not None and len(si.on_wait) > cap:
            waits = list(si.on_wait)
            extra, keep = waits[:-cap], waits[-cap:]
            for w in extra:
                _wsplit_counter[0] += 1
                nop = mybir.InstNoOp(
                    name=f"wsplit-{_wsplit_counter[0]}",
                    engine=inst.engine,
                    sync_info=mybir.SyncInfo(on_wait=[w], on_update=[]),
                    bass_nofuse=True,
                )
                nc.register_instruction(nop)
                new_list.append(nop)
            inst.sync_info = mybir.SyncInfo(on_wait=keep, on_update=list(si.on_update))
        new_list.append(inst)
    insts[:] = new_list


def _patched_lower(self, ordered):
    for insts in ordered.values():
        _split_waits_in_place(self.nc, insts)
    return _orig_lower(self, ordered)


def _patched_drain_and_barrier(self, tick_clock, wait_clock):
    nc = self.nc
    drain_inst = nc.sync.drain()
    wait_clock.add_sem_waits(
        drain_inst.ins, ScopedClock({None: tick_clock.global_clock})
    )
    si = drain_inst.ins.sync_info
    if si is not None and len(si.on_wait) > 1:
        waits = list(si.on_wait)
        drain_inst.ins.sync_info = _br.SyncInfo(
            on_wait=[waits[0]], on_update=list(si.on_update)
        )
        for w in waits[1:]:
            extra = nc.sync.drain()
            extra.ins.sync_info = _br.SyncInfo(on_wait=[w], on_update=[])
    nc.all_engine_barrier()
    assert self.sems is not None
    popped = nc._tile_sem_poison_stack.pop()
    assert popped is self._sem_poison
    nc.clear_and_free_semaphores(list(self.sems.allocated().values()))
    nc.all_engine_barrier()


tile.TileContext._lower_ordered_insts = _patched_lower
tile.TileContext._drain_and_barrier = _patched_drain_and_barrier

# ---------------------------------------------------------------------------
# NTFF profiling hook shim (the agent image's antenv lacks axon_hooks; the
# hook machinery itself is present in trn_agent_boot).  Only used when
# tracing is requested.
# ---------------------------------------------------------------------------
def _install_ntff_shim():
    if "antenv.axon_hooks" in sys.modules:
        return
    try:
        from trn_agent_boot.trn_boot import _ntff_profile_via_ctypes

        hook = _ntff_profile_via_ctypes("/opt/axon/libaxon_pjrt.so")
    except Exception:
        hook = None
    m = types.ModuleType("antenv.axon_hooks")
    m.get_axon_ntff_profile_hook = lambda: hook
    m.set_axon_ntff_profile_hook = lambda h: None
    import antenv  # noqa: F401

    sys.modules["antenv.axon_hooks"] = m


# ---------------------------------------------------------------------------
# Problem shape (hardcoded per contract)
# ---------------------------------------------------------------------------
M, K, N_TOTAL = 1024, 4096, 11008
NCORES = 8
N_LOC = N_TOTAL // NCORES  # 1376 unpacked columns per core
NP = N_LOC // 8            # 172 packed int32 columns per core
G = 32                     # scale/zero groups (group_size 128 == k-tile)
KT = K // 128              # 32 k-tiles
MT = M // 128              # 8 m-tiles
NB = 4                     # pair-blocks per core (one per unpack shift)
BW = N_LOC // NB           # 344 columns per pair-block / PSUM bank

PAIR_SHIFTS = (0, 4, 8, 12)
PAIR_MASK = 0x000F000F

F32 = mybir.dt.float32
BF16 = mybir.dt.bfloat16
I32 = mybir.dt.int32
I16 = mybir.dt.int16

LAST_EXEC_NS = None
LAST_TRACE = None

_cached_nc = None


def _bcast_row(ap_row, parts):
    """AP reading one DRAM row replicated across `parts` partitions."""
    return bass.AP(
        tensor=ap_row.tensor,
        offset=ap_row.offset,
        ap=[[0, parts]] + list(ap_row.ap[1:]),
    )


def _build():
    nc = bass.Bass()
    x_d = nc.declare_dram_parameter("x", [M, K], F32, isOutput=False)
    qw_d = nc.declare_dram_parameter("qw", [K, NP], I32, isOutput=False)
    sp_d = nc.declare_dram_parameter("sp", [G, N_LOC], BF16, isOutput=False)
    qz_d = nc.declare_dram_parameter("qz", [G, NP], I32, isOutput=False)
    out_d = nc.declare_dram_parameter("out", [M, N_LOC], F32, isOutput=True)

    AND = mybir.AluOpType.bitwise_and
    LSR = mybir.AluOpType.logical_shift_right
    MUL = mybir.AluOpType.mult

    with tile.TileContext(nc) as tc:
        from contextlib import ExitStack

        with ExitStack() as ctx:
            big = ctx.enter_context(tc.tile_pool(name="big", bufs=1))
            xT = big.tile([128, KT, M], BF16)        # x transposed, k on partitions
            W = big.tile([128, KT, N_LOC], BF16)     # dequant (w4*s), perm order

            consts = ctx.enter_context(tc.tile_pool(name="consts", bufs=1))
            # one-hot bank for the group-sum trick: onehot[:, 31] == 1
            onehot = consts.tile([128, 63], BF16)
            nc.vector.memset(onehot, 0.0)
            nc.vector.memset(onehot[:, 31:32], 1.0)

            sp_sb = consts.tile([G, N_LOC], BF16)
            nc.scalar.dma_start(out=sp_sb, in_=sp_d[:, :])
            qz_sb = consts.tile([G, NP], I32)
            nc.scalar.dma_start(out=qz_sb, in_=qz_d[:, :])
            znib = consts.tile([G, NB, NP], I32)
            for t in range(NB):
                nc.vector.tensor_scalar(
                    out=znib[:, t, :], in0=qz_sb,
                    scalar1=PAIR_SHIFTS[t], scalar2=PAIR_MASK,
                    op0=LSR, op1=AND,
                )
            z16 = znib.bitcast(I16).rearrange("p a b -> p (a b)")  # [G, N_LOC]
            ztmp = consts.tile([G, N_LOC], F32)
            nc.vector.tensor_tensor(out=ztmp, in0=z16, in1=sp_sb, op=MUL)
            B_bf = consts.tile([G, N_LOC], BF16)   # B = -(z*s), perm order
            nc.vector.tensor_scalar_mul(B_bf, ztmp, -1.0)

            xsumT = consts.tile([G, M], BF16)      # per-group column sums of x

            wprep = ctx.enter_context(tc.tile_pool(name="wprep", bufs=2))
            xprep = ctx.enter_context(tc.tile_pool(name="xprep", bufs=2))
            opool = ctx.enter_context(tc.tile_pool(name="oout", bufs=2))

            def w_prep(kt):
                q_sb = wprep.tile([128, NP], I32, tag="q")
                nc.scalar.dma_start(
                    out=q_sb, in_=qw_d[kt * 128:(kt + 1) * 128, :]
                )
                nib = wprep.tile([128, NB, NP], I32, tag="nib")
                for t in range(NB):
                    nc.vector.tensor_scalar(
                        out=nib[:, t, :], in0=q_sb,
                        scalar1=PAIR_SHIFTS[t], scalar2=PAIR_MASK,
                        op0=LSR, op1=AND,
                    )
                s_bc = wprep.tile([128, N_LOC], BF16, tag="sbc")
                nc.gpsimd.dma_start(
                    out=s_bc, in_=_bcast_row(sp_d[kt:kt + 1, :], 128)
                )
                nib16 = nib.bitcast(I16).rearrange("p a b -> p (a b)")
                nc.vector.tensor_tensor(
                    out=W[:, kt, :], in0=nib16, in1=s_bc, op=MUL
                )

            def x_prep(mt, ps32):
                x_bf = xprep.tile([128, K], BF16, tag="xbf")
                # SWDGE casting DMA: f32 DRAM -> bf16 SBUF
                nc.gpsimd.dma_start(
                    out=x_bf, in_=x_d[mt * 128:(mt + 1) * 128, :]
                )
                msl = slice(mt * 128, (mt + 1) * 128)
                for k2 in range(KT):
                    nc.sync.dma_start(
                        out=xT[:, k2, msl],
                        in_=x_bf[:, k2 * 128:(k2 + 1) * 128],
                        transpose=True,
                    )
                for k2 in range(KT):
                    nc.tensor.matmul(
                        ps32[:, msl],
                        lhsT=onehot[:, 31 - k2:63 - k2],
                        rhs=xT[:, k2, msl],
                        start=(k2 == 0), stop=(k2 == KT - 1),
                        skip_group_check=True,
                    )

            def run_chunk(mts, pc):
                ps = {
                    m: [pc.tile([128, BW], F32, name=f"ps_{m}_{t}", tag=f"pb{m % 2}_{t}") for t in range(NB)]
                    for m in mts
                }
                for kt in range(KT):
                    for m in mts:
                        for t in range(NB):
                            nc.tensor.matmul(
                                ps[m][t],
                                lhsT=xT[:, kt, m * 128:(m + 1) * 128],
                                rhs=W[:, kt, t * BW:(t + 1) * BW],
                                start=(kt == 0), stop=False,
                                skip_group_check=True,
                            )
                for m in mts:
                    for t in range(NB):
                        nc.tensor.matmul(
                            ps[m][t],
                            lhsT=xsumT[:, m * 128:(m + 1) * 128],
                            rhs=B_bf[:, t * BW:(t + 1) * BW],
                            start=False, stop=True,
                            skip_group_check=True,
                        )
                    out_sb = opool.tile([128, N_LOC], F32, tag="osb")
                    o3 = out_sb.rearrange("p (c j) -> p c j", j=8)
                    for t in range(NB):
                        src = ps[m][t].rearrange("p (c r) -> p c r", r=2)
                        nc.vector.tensor_copy(o3[:, :, 2 * t:2 * t + 2], src)
                    nc.scalar.dma_start(
                        out=out_d[m * 128:(m + 1) * 128, :], in_=out_sb
                    )

            with tc.tile_pool(name="ps32", bufs=1, space="PSUM") as pp32:
                ps32 = pp32.tile([G, M], F32)  # [32, 1024] fp32 = 2 banks
                for kt in range(KT):
                    w_prep(kt)
                    if kt % 4 == 0:
                        x_prep(kt // 4, ps32)
                nc.vector.tensor_copy(xsumT, ps32)
                # chunks 0/1 fit beside ps32 in PSUM (4 + 4 + 2 banks)
                for mts in ([0], [1]):
                    with tc.tile_pool(
                        name=f"psc{mts[0]}", bufs=1, space="PSUM"
                    ) as pc:
                        run_chunk(mts, pc)
            for mts in ([2, 3], [4, 5], [6, 7]):
                with tc.tile_pool(
                    name=f"psc{mts[0]}", bufs=1, space="PSUM"
                ) as pc:
                    run_chunk(mts, pc)

    return nc


def _get_nc():
    global _cached_nc
    if _cached_nc is None:
        _cached_nc = _build()
    return _cached_nc


def kernel(x, qweight, scales, qzeros):
    global LAST_EXEC_NS, LAST_TRACE

    x = np.ascontiguousarray(np.asarray(x, dtype=np.float32))
    qweight = np.asarray(qweight, dtype=np.int32)
    scales = np.asarray(scales, dtype=np.float32)
    qzeros = np.asarray(qzeros, dtype=np.int32)

    in_maps = []
    for c in range(NCORES):
        qw_c = np.ascontiguousarray(qweight[:, c * NP:(c + 1) * NP])
        qz_c = np.ascontiguousarray(qzeros[:, c * NP:(c + 1) * NP])
        s_c = scales[:, c * N_LOC:(c + 1) * N_LOC]
        # pair-block permutation: dest[g, 344*t + 2*cc + r] = s[g, 8*cc + 2*t + r]
        s_perm = np.ascontiguousarray(
            s_c.reshape(G, NP, 4, 2).transpose(0, 2, 1, 3).reshape(G, N_LOC)
        ).astype(ml_dtypes.bfloat16)
        in_maps.append({"x": x, "qw": qw_c, "sp": s_perm, "qz": qz_c})

    trace = os.environ.get("AWQ_KERNEL_TRACE", "0") == "1"
    if trace:
        _install_ntff_shim()

    nc = _get_nc()
    res = run_bass_kernel_spmd(
        nc, in_maps, core_ids=list(range(NCORES)), trace=trace
    )
    LAST_EXEC_NS = res.exec_time_ns
    if res.instructions_and_trace is not None:
        LAST_TRACE = res.instructions_and_trace[1]

    return np.concatenate(
        [res.results[i]["out"] for i in range(NCORES)], axis=1
    )
